# revision 1
# baseline (speedup 1.0000x reference)
"""RelGraphConvOps forward on 8 TRN2 NeuronCores (Bass/Tile).

Strategy: shard edges by DST node across cores (12500 dst rows each) -> no
cross-core reduction needed. Per core:
  - dma_gather (bf16, 256B rows) of feat[src] from a [feat|feat] table,
    grouped by (src bank of 32768, dst stripe of 128).
  - per-edge basis scaling msgs2 = [c0*feat | c1*feat] via one batched DVE mul.
  - segment-sum onto dst via one-hot PE matmuls: lhsT = (iota==dstloc) bf16,
    rhs = msgs2; accumulate per (bank,stripe) run in PSUM (f32), add into a
    SBUF-resident agg[12500, 128].
  - dense finish: h.T = Wcat.T @ agg.T + loopW.T @ feat.T (+bias, relu) with
    PE transposes, per 128-dst stripe.
"""
import sys
for p in ('/opt/trn_rl_repo', '/root/.axon_site/_ro/trn_rl_repo'):
    if p not in sys.path:
        sys.path.insert(0, p)

import numpy as np
import ml_dtypes

import concourse.bacc as bacc
import concourse.mybir as mybir
import concourse.tile as tile
from concourse.bass_utils import run_bass_kernel_spmd

BF16 = mybir.dt.bfloat16
F32 = mybir.dt.float32
I16 = mybir.dt.int16

N = 100000
E = 1600000
F = 64
NREL = 8
NBASis = 2
CORES = 8
SHARD = N // CORES            # 12500
STRIPE = 128
NSTRIPE = (SHARD + STRIPE - 1) // STRIPE   # 98
BANKROWS = 32768
NBANKS = (N + BANKROWS - 1) // BANKROWS    # 4
TCAP = 1024                   # max gather tokens per instruction (SWDGE ring)
PAD_DLOC = 255.0


def _wrap_idx16(idx):
    """int16 idx stream -> [128, n/16] (16-partition wrap, replicated x8)."""
    a = np.asarray(idx, dtype=np.int16).reshape(-1, 16).T   # [16, n/16]
    return np.tile(a, (8, 1)).copy()


def _host_prep(feat, coeff, src, dst, et):
    """Returns (shared_inputs, per_core_inputs, cols_meta, chunks_meta)."""
    src = np.asarray(src).astype(np.int64)
    dst = np.asarray(dst).astype(np.int64)
    et = np.asarray(et).astype(np.int64)

    core = dst // SHARD
    bank = src >> 15
    srcloc = src & (BANKROWS - 1)
    dlocal = dst % SHARD
    stripe = dlocal >> 7
    dloc = dlocal & (STRIPE - 1)
    c0 = coeff[et, 0].astype(np.float32)
    c1 = coeff[et, 1].astype(np.float32)

    # counts[core, bank, stripe]
    key_bs = bank * NSTRIPE + stripe
    counts = np.zeros((CORES, NBANKS * NSTRIPE), np.int64)
    for c in range(CORES):
        m = core == c
        counts[c] = np.bincount(key_bs[m], minlength=NBANKS * NSTRIPE)
    ncol_run = (counts.max(axis=0) + 127) // 128          # [NBANKS*NSTRIPE]

    # column metadata (bank-major stream), chunk boundaries within banks
    cols = []      # (stripe, run_start, run_end)
    chunks = []    # dict(bank, slot_lo, nslots, col_lo, ncols)
    col_base = 0
    for b in range(NBANKS):
        bank_col_lo = col_base
        for s in range(NSTRIPE):
            nc_ = int(ncol_run[b * NSTRIPE + s])
            for k in range(nc_):
                cols.append((s, k == 0, k == nc_ - 1))
            col_base += nc_
        # chunk this bank's columns
        c_lo = bank_col_lo
        while c_lo < col_base:
            cn = min(TCAP // 128, col_base - c_lo)
            chunks.append(dict(bank=b, col_lo=c_lo, ncols=cn))
            c_lo += cn
    ncol_tot = col_base
    nslots = ncol_tot * 128

    # per-core slot arrays
    run_col_lo = np.zeros(NBANKS * NSTRIPE, np.int64)
    acc = 0
    for i in range(NBANKS * NSTRIPE):
        run_col_lo[i] = acc
        acc += ncol_run[i]

    per_core = []
    for c in range(CORES):
        m = np.where(core == c)[0]
        k = key_bs[m]
        order = np.argsort(k, kind='stable')
        me = m[order]
        ks = k[order]
        # slot position: run base + offset within run
        run_off = np.arange(len(me)) - np.searchsorted(ks, ks, side='left')
        slot = run_col_lo[ks] * 128 + run_off

        gidx = np.zeros(nslots, np.int16)
        dl = np.full(nslots, PAD_DLOC, np.float32)
        cc0 = np.zeros(nslots, np.float32)
        cc1 = np.zeros(nslots, np.float32)
        gidx[slot] = srcloc[me]
        dl[slot] = dloc[me]
        cc0[slot] = c0[me]
        cc1[slot] = c1[me]

        # device layouts
        # token slot i -> (p=i%128, col=i//128)
        dst_sb = dl.reshape(ncol_tot, 128).T.astype(ml_dtypes.bfloat16)   # [128, NCOL]
        c01 = np.stack([cc0, cc1], axis=-1).reshape(ncol_tot, 128, 2)
        c01_sb = c01.transpose(1, 0, 2).astype(ml_dtypes.bfloat16)        # [128, NCOL, 2]
        per_core.append(dict(
            gidx=_wrap_idx16(gidx),
            dstv=np.ascontiguousarray(dst_sb),
            c01v=np.ascontiguousarray(c01_sb),
            feat32=np.ascontiguousarray(feat[c * SHARD:(c + 1) * SHARD]).astype(np.float32),
        ))
    return per_core, cols, chunks, ncol_tot, nslots


def _build_program(cols, chunks, ncol_tot, nslots):
    nc = bacc.Bacc("TRN2", target_bir_lowering=False, debug=False)

    table_d = nc.dram_tensor("table", [N, 2 * F], BF16, kind="ExternalInput")
    gidx_d = nc.dram_tensor("gidx", [128, nslots // 16], I16, kind="ExternalInput")
    dstv_d = nc.dram_tensor("dstv", [128, ncol_tot], BF16, kind="ExternalInput")
    c01v_d = nc.dram_tensor("c01v", [128, ncol_tot, 2], BF16, kind="ExternalInput")
    iota_d = nc.dram_tensor("iota", [128, 128], BF16, kind="ExternalInput")
    ident_d = nc.dram_tensor("ident", [128, 128], F32, kind="ExternalInput")
    feat_d = nc.dram_tensor("feat32", [SHARD, F], F32, kind="ExternalInput")
    wcat_d = nc.dram_tensor("wcat", [2 * F, F], F32, kind="ExternalInput")
    loopw_d = nc.dram_tensor("loopw", [F, F], F32, kind="ExternalInput")
    bias_d = nc.dram_tensor("bias", [F, 1], F32, kind="ExternalInput")
    out_d = nc.dram_tensor("out", [SHARD, F], F32, kind="ExternalOutput")

    bank_lims = [(b * BANKROWS, min(N, (b + 1) * BANKROWS)) for b in range(NBANKS)]

    with tile.TileContext(nc) as tc:
        with (
            tc.tile_pool(name="const", bufs=1) as constp,
            tc.tile_pool(name="aggp", bufs=1) as aggp,
            tc.tile_pool(name="gip", bufs=3) as gip,
            tc.tile_pool(name="msgp", bufs=3) as msgp,
            tc.tile_pool(name="msgs2p", bufs=2) as msgs2p,
            tc.tile_pool(name="t1p", bufs=2) as t1p,
            tc.tile_pool(name="psrun", bufs=2, space="PSUM") as psrun,
            tc.tile_pool(name="pst", bufs=2, space="PSUM") as pst,
            tc.tile_pool(name="psft", bufs=1, space="PSUM") as psft,
            tc.tile_pool(name="psh", bufs=2, space="PSUM") as psh,
            tc.tile_pool(name="pso", bufs=1, space="PSUM") as pso,
            tc.tile_pool(name="finp", bufs=2) as finp,
        ):
            iota_sb = constp.tile([128, 128], BF16)
            nc.sync.dma_start(iota_sb[:], iota_d[:])
            ident_sb = constp.tile([128, 128], F32)
            nc.sync.dma_start(ident_sb[:], ident_d[:])
            dst_sb = constp.tile([128, ncol_tot], BF16)
            nc.sync.dma_start(dst_sb[:], dstv_d[:])
            c01_sb = constp.tile([128, ncol_tot, 2], BF16)
            nc.sync.dma_start(c01_sb[:], c01v_d[:])
            wcat_sb = constp.tile([2 * F, F], F32)
            nc.sync.dma_start(wcat_sb[:], wcat_d[:])
            loop_sb = constp.tile([F, F], F32)
            nc.sync.dma_start(loop_sb[:], loopw_d[:])
            bias_sb = constp.tile([F, 1], F32)
            nc.sync.dma_start(bias_sb[:], bias_d[:])

            agg = aggp.tile([128, NSTRIPE * 128], F32)
            nc.vector.memset(agg[:], 0.0)

            # ---- main phase ----
            ps = None
            for ch in chunks:
                b = ch["bank"]
                row_lo, row_hi = bank_lims[b]
                col_lo, ncols = ch["col_lo"], ch["ncols"]
                slots = ncols * 128

                gi = gip.tile([128, TCAP // 16], I16, tag="gi")
                nc.sync.dma_start(
                    gi[:, : slots // 16],
                    gidx_d[:, col_lo * 8: col_lo * 8 + slots // 16])
                msg = msgp.tile([128, TCAP // 128, 2 * F], BF16, tag="msg")
                nc.gpsimd.dma_gather(
                    msg[:, :ncols, :], table_d[row_lo:row_hi, :],
                    gi[:, : slots // 16], slots, slots, 2 * F)

                msgs2 = msgs2p.tile([128, TCAP // 128, 2 * F], BF16, tag="m2")
                in0 = msg[:, :ncols, :].rearrange("p c (b f) -> p c b f", b=2)
                in1 = (c01_sb[:, col_lo:col_lo + ncols, :]
                       .rearrange("p c (b o) -> p c b o", o=1)
                       .broadcast_to([128, ncols, 2, F]))
                nc.vector.tensor_tensor(
                    msgs2[:, :ncols, :].rearrange("p c (b f) -> p c b f", b=2),
                    in0, in1, mybir.AluOpType.mult)

                t1 = t1p.tile([128, TCAP // 128, 128], BF16, tag="t1")
                i0 = (iota_sb[:].rearrange("p (o j) -> p o j", o=1)
                      .broadcast_to([128, ncols, 128]))
                i1 = (dst_sb[:, col_lo:col_lo + ncols]
                      .rearrange("p (c o) -> p c o", o=1)
                      .broadcast_to([128, ncols, 128]))
                nc.vector.tensor_tensor(
                    t1[:, :ncols, :], i0, i1, mybir.AluOpType.is_equal)

                for ci in range(ncols):
                    s, r_start, r_end = cols[col_lo + ci]
                    if r_start:
                        ps = psrun.tile([128, 128], F32, tag="ps")
                    nc.tensor.matmul(ps[:], t1[:, ci, :], msgs2[:, ci, :],
                                     start=r_start, stop=r_end)
                    if r_end:
                        nc.vector.tensor_tensor(
                            agg[:, s * 128:(s + 1) * 128],
                            agg[:, s * 128:(s + 1) * 128],
                            ps[:], mybir.AluOpType.add)

            # ---- final phase ----
            for s in range(NSTRIPE):
                rows = min(STRIPE, SHARD - s * STRIPE)
                pt = pst.tile([128, 128], F32, tag="pt")
                nc.tensor.transpose(pt[:], agg[:, s * 128:(s + 1) * 128],
                                    ident_sb[:])
                aggT = finp.tile([128, 128], F32, tag="aggT")
                nc.vector.tensor_copy(aggT[:], pt[:])

                ft = finp.tile([128, F], F32, tag="ft")
                if rows < 128:
                    nc.vector.memset(ft[:], 0.0)
                nc.sync.dma_start(ft[:rows, :],
                                  feat_d[s * STRIPE: s * STRIPE + rows, :])
                pft = psft.tile([F, 128], F32, tag="pft")
                nc.tensor.transpose(pft[:], ft[:], ident_sb[:])
                featT = finp.tile([F, 128], F32, tag="featT")
                nc.vector.tensor_copy(featT[:], pft[:])

                ph = psh.tile([F, 128], F32, tag="ph")
                nc.tensor.matmul(ph[:], wcat_sb[:], aggT[:],
                                 start=True, stop=False)
                nc.tensor.matmul(ph[:], loop_sb[:], featT[:],
                                 start=False, stop=True)
                hT = finp.tile([F, 128], F32, tag="hT")
                nc.scalar.activation(hT[:], ph[:],
                                     mybir.ActivationFunctionType.Relu,
                                     bias=bias_sb[:, 0:1], scale=1.0)
                po = pso.tile([128, F], F32, tag="po")
                nc.tensor.transpose(po[:], hT[:], ident_sb[:F, :F])
                h = finp.tile([128, F], F32, tag="h")
                nc.vector.tensor_copy(h[:], po[:])
                nc.sync.dma_start(out_d[s * STRIPE: s * STRIPE + rows, :],
                                  h[:rows, :])
    nc.compile()
    return nc


def kernel(feat, coeff, W, h_bias, loop_weight, src_ids, dst_ids, etypes):
    feat = np.asarray(feat, np.float32)
    coeff = np.asarray(coeff, np.float32)
    W = np.asarray(W, np.float32)
    h_bias = np.asarray(h_bias, np.float32)
    loop_weight = np.asarray(loop_weight, np.float32)

    per_core, cols, chunks, ncol_tot, nslots = _host_prep(
        feat, coeff, src_ids, dst_ids, etypes)

    nc = _build_program(cols, chunks, ncol_tot, nslots)

    feat_bf = feat.astype(ml_dtypes.bfloat16)
    table = np.concatenate([feat_bf, feat_bf], axis=1)          # [N, 128]
    iota = np.tile(np.arange(128, dtype=np.float32), (128, 1)).astype(
        ml_dtypes.bfloat16)
    ident = np.eye(128, dtype=np.float32)
    wcat = W.reshape(2 * F, F)
    bias = h_bias.reshape(F, 1)

    shared = dict(table=table, iota=iota, ident=ident, wcat=wcat,
                  loopw=loop_weight, bias=bias)
    in_maps = []
    for c in range(CORES):
        m = dict(shared)
        m.update(per_core[c])
        in_maps.append(m)

    r = run_bass_kernel_spmd(nc, in_maps, core_ids=list(range(CORES)))
    out = np.concatenate([r.results[c]["out"] for c in range(CORES)], axis=0)
    return out.astype(np.float32)


if __name__ == "__main__":
    # smoke test with tiny random data is not possible (shapes hardcoded);
    # use test.py instead.
    pass
